# revision 15
# baseline (speedup 1.0000x reference)
"""Trainium2 Bass kernel for nn_EmbedMatcher (GNN message passing).

Strategy: data-parallel over B=1024 across 8 cores (128 rows each); the
200001x128 symbol table is replicated per core (plus a bf16 copy staged
for the gather path, halving queue bytes). The neighbor gather is 400
[P,1] indirect DMAs per core in bypass mode -- on this HW the SWDGE
engine generates one descriptor per gathered row at ~1.1us/128 rows,
which is the measured floor across every gather primitive (dma_gather's
Q7 path measured ~10.4ns/row and its int16 windowing forces ~60% pad
rows; multi-queue SWDGE corrupts results because concurrent queue pairs
collide on shared ucode state). bypass (vs compute_op=add) halves
descriptor work, and the K-sum rides the otherwise-idle Vector engine.
NCHAIN=24 accumulate buffers give the gather->add->gather WAR loop ~24
instructions of slack so the semaphore round-trip latency (~0.3us per
gather of issue gap at NCHAIN=8) stays off the SWDGE critical path.
The GCN linear is algebraically reordered: sum_k (concat @ W^T) ==
[rel_sum|ent_sum] @ W^T, so the big (B*K,256)@(256,128) matmul collapses
to (B,256)@(256,128). The loop-invariant q@W_ih gates and qT transpose
are computed inside the gather window. support_g is AllGathered in both
layouts ([sg | sgT], one local pre-transpose instead of 8
post-collective ones) so each core runs the LSTM attention (which
couples the whole batch) on its own 128 query rows; softmax skips
max-subtraction (scores are O(+-30), safe for f32 exp).

LSTM sigmoids are computed as 0.5*tanh(x/2)+0.5 folded into
scalar_tensor_tensor ops (same op count, cell state kept doubled), so
every activation in the loop lives in the single 'exp_and_others'
Scalar table (exp/tanh/square/relu) - the baseline reloaded activation
tables ~9 times (~11.5us) swapping between sigmoid and exp sets.
"""
import numpy as np

from concourse import bass, bacc, mybir
import concourse.tile as tile
from concourse.bass_utils import run_bass_kernel_spmd

P = 128            # batch rows per core
D = 128            # embed dim
K = 200            # neighbors
NCORES = 8
NROWS = 200001     # symbol table rows (incl. padding row)
NCHAIN = 24        # parallel accumulate chains per table
STEPS = 4
F32 = mybir.dt.float32
I32 = mybir.dt.int32

_CACHE = {}


def _build():
    nc = bacc.Bacc("TRN2", target_bir_lowering=False, debug=False,
                   enable_asserts=True, num_devices=NCORES)
    ap = {}
    def inp(name, shape, dtype=F32):
        ap[name] = nc.dram_tensor(name, shape, dtype, kind="ExternalInput").ap()
    inp("table", [NROWS, D])
    inp("table16", [NROWS, D], mybir.dt.bfloat16)
    inp("idx", [P, 2 * K], I32)
    inp("qidx", [P, 1], I32)
    inp("wrT", [D, D]); inp("weT", [D, D]); inp("gcnb", [P, D])
    inp("p1wT", [D, 2 * D]); inp("p1b", [P, 2])
    inp("p2wTa", [D, D]); inp("p2wTb", [D, D]); inp("p2b", [P, D])
    inp("lna", [P, D]); inp("lnb", [P, D])
    inp("wihT", [D, 8 * D]); inp("whhTa", [D, 8 * D]); inp("whhTb", [D, 8 * D])
    inp("gbias", [P, 8 * D])
    out_d = nc.dram_tensor("out", [P, 1], F32, kind="ExternalOutput").ap()

    from concourse.masks import make_identity
    AX = mybir.AxisListType.X
    OP = mybir.AluOpType
    ACT = mybir.ActivationFunctionType

    with tile.TileContext(nc, num_cores=NCORES) as tc:
        with tc.tile_pool(name="sb", bufs=1) as sb, \
             tc.tile_pool(name="ps", bufs=2, space="PSUM") as ps, \
             tc.tile_pool(name="pst", bufs=2, space="PSUM") as pst, \
             tc.tile_pool(name="dram", bufs=1, space="DRAM") as dram:

            ident = sb.tile([P, P], F32)
            make_identity(nc, ident[:])

            # ---- load inputs to SBUF
            idx_sb = sb.tile([P, 2 * K], I32)
            nc.sync.dma_start(out=idx_sb[:], in_=ap["idx"][:])
            qidx_sb = sb.tile([P, 1], I32)
            nc.sync.dma_start(out=qidx_sb[:], in_=ap["qidx"][:])
            w = {}
            for name, shape in [("wrT", [D, D]), ("weT", [D, D]), ("gcnb", [P, D]),
                                ("p1wT", [D, 2 * D]), ("p1b", [P, 2]),
                                ("p2wTa", [D, D]), ("p2wTb", [D, D]), ("p2b", [P, D]),
                                ("lna", [P, D]), ("lnb", [P, D]),
                                ("wihT", [D, 8 * D]), ("whhTa", [D, 8 * D]),
                                ("whhTb", [D, 8 * D]), ("gbias", [P, 8 * D])]:
                w[name] = sb.tile(shape, F32, name=f"w_{name}")
                nc.sync.dma_start(out=w[name][:], in_=ap[name][:])

            def transpose_to(dst_sb, src_ap, nm):
                tp = pst.tile([P, P], F32, name=f"tp_{nm}", tag="tp")
                nc.tensor.transpose(out=tp[:], in_=src_ap, identity=ident[:])
                nc.vector.tensor_copy(out=dst_sb, in_=tp[:])

            # ---- query gather (B,) -> (P, D)
            q_sb = sb.tile([P, D], F32)
            nc.gpsimd.indirect_dma_start(
                out=q_sb[:], out_offset=None, in_=ap["table"][:],
                in_offset=bass.IndirectOffsetOnAxis(ap=qidx_sb[:, 0:1], axis=0))

            # ---- qT and q-gates: loop-invariant, computed during the
            # gather phase while TensorE is otherwise idle
            qT = sb.tile([P, P], F32)
            transpose_to(qT[:], q_sb[:], "q")
            gq = sb.tile([P, 8 * D], F32, name="gq")
            for j in range(2):
                gp = ps.tile([P, 512], F32, name=f"gq{j}", tag="gates")
                sl = slice(512 * j, 512 * (j + 1))
                nc.tensor.matmul(out=gp[:], lhsT=qT[:], rhs=w["wihT"][:, sl],
                                 start=True, stop=True)
                nc.vector.tensor_add(out=gq[:, sl], in0=gp[:],
                                     in1=w["gbias"][:, sl])

            # ---- neighbor gathers: [P,1] indirect DMAs (bypass) + VectorE
            # accumulate. On real HW an indirect DMA reads only ONE index per
            # partition (multi-column offset APs silently stream consecutive
            # rows), so the shape is forced to [P,1]. bypass halves the SWDGE
            # descriptor-gen work vs compute_op=add chains (RMW emits read
            # descriptors too); the K-sum rides the idle Vector engine.
            sums = []
            BF16 = mybir.dt.bfloat16
            bufs = [sb.tile([P, D], BF16, name=f"gbuf{i}")
                    for i in range(NCHAIN)]
            for half in range(2):          # 0 = relations, 1 = entities
                s = sb.tile([P, D], F32, name=f"sum{half}")
                for t in range(K):
                    col = half * K + t
                    b = bufs[t % NCHAIN]
                    nc.gpsimd.indirect_dma_start(
                        out=b[:], out_offset=None, in_=ap["table16"][:],
                        in_offset=bass.IndirectOffsetOnAxis(
                            ap=idx_sb[:, col:col + 1], axis=0))
                    if t == 0:
                        nc.vector.tensor_copy(out=s[:], in_=b[:])
                    else:
                        nc.vector.tensor_add(out=s[:], in0=s[:], in1=b[:])
                sums.append(s)
            rel_sum, ent_sum = sums

            # ---- GCN: support = tanh((rel_sum@Wr' + ent_sum@We') + gcnb)
            relT = sb.tile([P, P], F32); transpose_to(relT[:], rel_sum[:], "rel")
            entT = sb.tile([P, P], F32); transpose_to(entT[:], ent_sum[:], "ent")
            sup_ps = ps.tile([P, D], F32, name="sup_ps", tag="mm")
            nc.tensor.matmul(out=sup_ps[:], lhsT=relT[:], rhs=w["wrT"][:],
                             start=True, stop=False)
            nc.tensor.matmul(out=sup_ps[:], lhsT=entT[:], rhs=w["weT"][:],
                             start=False, stop=True)
            support = sb.tile([P, D], F32)
            nc.vector.tensor_add(out=support[:], in0=sup_ps[:], in1=w["gcnb"][:])
            nc.scalar.activation(out=support[:], in_=support[:], func=ACT.Tanh)

            # ---- FFN + residual + layernorm -> support_g
            supT = sb.tile([P, P], F32); transpose_to(supT[:], support[:], "sup")
            hidT = []
            for j in range(2):
                hp = ps.tile([P, D], F32, name=f"hid_ps{j}", tag="mm")
                nc.tensor.matmul(out=hp[:], lhsT=w["p1wT"][:, j * D:(j + 1) * D],
                                 rhs=supT[:], start=True, stop=True)
                ht = sb.tile([P, P], F32, name=f"hidT{j}")
                nc.scalar.activation(out=ht[:], in_=hp[:], func=ACT.Relu,
                                     bias=w["p1b"][:, j:j + 1])
                hidT.append(ht)
            o2 = ps.tile([P, D], F32, name="o2", tag="mm")
            nc.tensor.matmul(out=o2[:], lhsT=hidT[0][:], rhs=w["p2wTa"][:],
                             start=True, stop=False)
            nc.tensor.matmul(out=o2[:], lhsT=hidT[1][:], rhs=w["p2wTb"][:],
                             start=False, stop=True)
            z = sb.tile([P, D], F32)
            nc.vector.tensor_add(out=z[:], in0=o2[:], in1=support[:])
            nc.vector.tensor_add(out=z[:], in0=z[:], in1=w["p2b"][:])
            # layernorm (unbiased std, eps added to std)
            zsum = sb.tile([P, 1], F32)
            nc.vector.tensor_reduce(out=zsum[:], in_=z[:], axis=AX, op=OP.add)
            zmean = sb.tile([P, 1], F32)
            nc.scalar.mul(out=zmean[:], in_=zsum[:], mul=1.0 / D)
            xc = sb.tile([P, D], F32)
            nc.vector.tensor_scalar(out=xc[:], in0=z[:], scalar1=zmean[:, 0:1],
                                    scalar2=None, op0=OP.subtract)
            sqt = sb.tile([P, D], F32)
            varsum = sb.tile([P, 1], F32)
            nc.scalar.activation(out=sqt[:], in_=xc[:], func=ACT.Square,
                                 accum_out=varsum[:])
            sigma = sb.tile([P, 1], F32)
            nc.scalar.activation(out=sigma[:], in_=varsum[:], func=ACT.Sqrt,
                                 scale=1.0 / (D - 1))
            nc.vector.tensor_scalar(out=sigma[:], in0=sigma[:], scalar1=1e-3,
                                    scalar2=None, op0=OP.add)
            rec = sb.tile([P, 1], F32)
            nc.vector.reciprocal(out=rec[:], in_=sigma[:])
            sg = sb.tile([P, D], F32)
            nc.vector.tensor_scalar(out=sg[:], in0=xc[:], scalar1=rec[:, 0:1],
                                    scalar2=None, op0=OP.mult)
            nc.vector.tensor_tensor(out=sg[:], in0=sg[:], in1=w["lna"][:],
                                    op=OP.mult)
            nc.vector.tensor_tensor(out=sg[:], in0=sg[:], in1=w["lnb"][:],
                                    op=OP.add)

            # ---- AllGather [sg | sgT] -> both layouts on every core
            # (one local transpose pre-AG replaces 8 post-collective ones)
            sgT_own = sb.tile([P, P], F32)
            transpose_to(sgT_own[:], sg[:], "sgown")
            ag_in = dram.tile([P, 2 * D], F32)
            ag_out = dram.tile([NCORES * P, 2 * D], F32)
            nc.gpsimd.dma_start(out=ag_in[:, 0:D], in_=sg[:])
            nc.gpsimd.dma_start(out=ag_in[:, D:2 * D], in_=sgT_own[:])
            nc.gpsimd.collective_compute(
                "AllGather", OP.bypass,
                replica_groups=[list(range(NCORES))],
                ins=[ag_in.opt()], outs=[ag_out.opt()])
            sg_all = sb.tile([P, NCORES, D], F32)
            nc.sync.dma_start(
                out=sg_all[:],
                in_=ag_out[:, 0:D].rearrange("(c p) d -> p c d", c=NCORES))
            sgT3 = sb.tile([P, NCORES, P], F32)
            nc.sync.dma_start(
                out=sgT3[:],
                in_=ag_out[:, D:2 * D].rearrange("(c d) b -> d c b",
                                                 c=NCORES))
            sgT = sgT3[:].rearrange("d c b -> d (c b)")

            # ---- LSTM + attention. sigmoid(x) = 0.5*tanh(x/2)+0.5; the
            # 0.5s are folded into scalar_tensor_tensor ops and the cell
            # state is kept DOUBLED (C = 2c) so every activation stays in
            # the exp_and_others table (no in-loop table reloads).
            c2 = sb.tile([P, 2 * D], F32)       # doubled cell state
            gts = sb.tile([P, 8 * D], F32)
            ti = sb.tile([P, 2 * D], F32)
            tf = sb.tile([P, 2 * D], F32)
            tg = sb.tile([P, 2 * D], F32)
            to = sb.tile([P, D], F32)
            tch = sb.tile([P, D], F32)
            tmp = sb.tile([P, 2 * D], F32)
            h2 = sb.tile([P, D], F32)
            ho = sb.tile([P, D], F32)
            hoT = sb.tile([P, P], F32)
            rT_sb = sb.tile([P, P], F32)
            attn = sb.tile([P, NCORES * P], F32)
            rowsum = sb.tile([P, 1], F32)
            rsrec = sb.tile([P, 1], F32)

            for s in range(STEPS):
                if s == 0:
                    gsrc = gq
                else:
                    gsrc = gts
                    for j in range(2):  # gate halves: g-slices [512j, 512j+512)
                        gp = ps.tile([P, 512], F32, name=f"g{s}{j}", tag="gates")
                        sl = slice(512 * j, 512 * (j + 1))
                        nc.tensor.matmul(out=gp[:], lhsT=hoT[:],
                                         rhs=w["whhTa"][:, sl],
                                         start=True, stop=False)
                        nc.tensor.matmul(out=gp[:], lhsT=rT_sb[:],
                                         rhs=w["whhTb"][:, sl],
                                         start=False, stop=True)
                        nc.vector.tensor_add(out=gts[:, sl], in0=gp[:],
                                             in1=gq[:, sl])
                # i,f,g,o = gsrc[0:256],[256:512],[512:768],[768:1024]
                # ti = tanh(i/2)  (sig(i) = 0.5*(ti+1))
                nc.scalar.activation(out=ti[:], in_=gsrc[:, 0:256],
                                     func=ACT.Tanh, scale=0.5)
                nc.scalar.activation(out=tg[:], in_=gsrc[:, 512:768],
                                     func=ACT.Tanh)
                nc.scalar.activation(out=to[:], in_=gsrc[:, 768:896],
                                     func=ACT.Tanh, scale=0.5)
                # X = (ti+1)*tg = 2*sig(i)*tanh(g)
                nc.vector.scalar_tensor_tensor(
                    out=tmp[:], in0=ti[:], scalar=1.0, in1=tg[:],
                    op0=OP.add, op1=OP.mult)
                if s == 0:
                    nc.vector.tensor_copy(out=c2[:], in_=tmp[:])
                else:
                    nc.scalar.activation(out=tf[:], in_=gsrc[:, 256:512],
                                         func=ACT.Tanh, scale=0.5)
                    # Y = (tf+1)*C = 4*sig(f)*c ; C' = 0.5*Y + X = 2*c'
                    nc.vector.scalar_tensor_tensor(
                        out=tf[:], in0=tf[:], scalar=1.0, in1=c2[:],
                        op0=OP.add, op1=OP.mult)
                    nc.vector.scalar_tensor_tensor(
                        out=c2[:], in0=tf[:], scalar=0.5, in1=tmp[:],
                        op0=OP.mult, op1=OP.add)
                # tanh(c) = tanh(0.5*C)
                nc.scalar.activation(out=tch[:], in_=c2[:, 0:D],
                                     func=ACT.Tanh, scale=0.5)
                # h = sig(o)*tanh(c) = 0.5*(to+1)*tch ; ho = q + h
                nc.vector.scalar_tensor_tensor(
                    out=h2[:], in0=to[:], scalar=1.0, in1=tch[:],
                    op0=OP.add, op1=OP.mult)
                nc.vector.scalar_tensor_tensor(
                    out=ho[:], in0=h2[:], scalar=0.5, in1=q_sb[:],
                    op0=OP.mult, op1=OP.add)
                if s == STEPS - 1:
                    break
                transpose_to(hoT[:], ho[:], f"ho{s}")
                sc = ps.tile([P, NCORES * P], F32, name=f"sc{s}", tag="scores",
                             bufs=1)
                for j in range(2):
                    nc.tensor.matmul(out=sc[:, 512 * j:512 * (j + 1)],
                                     lhsT=hoT[:],
                                     rhs=sgT[:, 512 * j:512 * (j + 1)],
                                     start=True, stop=True)
                # scores are O(+-30): exp is safe in f32 without the
                # max-subtraction, saving a 1024-wide reduce per step
                nc.scalar.activation(out=attn[:], in_=sc[:], func=ACT.Exp,
                                     accum_out=rowsum[:])
                nc.vector.reciprocal(out=rsrec[:], in_=rowsum[:])
                nc.vector.tensor_scalar(out=attn[:], in0=attn[:],
                                        scalar1=rsrec[:, 0:1], scalar2=None,
                                        op0=OP.mult)
                rp = ps.tile([P, D], F32, name=f"rp{s}", tag="mm")
                for c in range(NCORES):
                    at = sb.tile([P, P], F32, name=f"at{s}{c}", tag="atT",
                                 bufs=2)
                    transpose_to(at[:], attn[:, c * P:(c + 1) * P], f"at{s}{c}")
                    nc.tensor.matmul(out=rp[:], lhsT=sg_all[:, c, :], rhs=at[:],
                                     start=(c == 0), stop=(c == NCORES - 1))
                nc.vector.tensor_copy(out=rT_sb[:], in_=rp[:])

            # ---- cosine similarity against own support_g shard
            m1 = sb.tile([P, D], F32)
            nc.vector.tensor_tensor(out=m1[:], in0=ho[:], in1=sg[:], op=OP.mult)
            cross = sb.tile([P, 1], F32)
            nc.vector.tensor_reduce(out=cross[:], in_=m1[:], axis=AX, op=OP.add)
            n1 = sb.tile([P, 1], F32)
            n2 = sb.tile([P, 1], F32)
            nc.scalar.activation(out=m1[:], in_=ho[:], func=ACT.Square,
                                 accum_out=n1[:])
            nc.scalar.activation(out=m1[:], in_=sg[:], func=ACT.Square,
                                 accum_out=n2[:])
            nc.vector.tensor_tensor(out=n1[:], in0=n1[:], in1=n2[:], op=OP.mult)
            nc.scalar.activation(out=n1[:], in_=n1[:], func=ACT.Sqrt)
            nc.vector.reciprocal(out=n1[:], in_=n1[:])
            res = sb.tile([P, 1], F32)
            nc.vector.tensor_tensor(out=res[:], in0=cross[:], in1=n1[:],
                                    op=OP.mult)
            nc.sync.dma_start(out=out_d[:], in_=res[:])
    nc.compile()
    return nc


def _prep_inputs(relations, entities, query, symbol_emb, gcn_w_w, gcn_w_b,
                 p1_w, p1_b, p2_w, p2_b, ln_a, ln_b, w_ih, w_hh, b_ih, b_hh):
    import ml_dtypes
    f32 = np.float32
    table = np.ascontiguousarray(symbol_emb, dtype=f32)
    table16 = table.astype(ml_dtypes.bfloat16)
    B = relations.shape[0]
    rel = np.asarray(relations).astype(np.int32)
    ent = np.asarray(entities).astype(np.int32)
    qry = np.asarray(query).astype(np.int32).reshape(B, 1)
    inv = f32(1.0 / B)                     # reference divides by B (quirk)
    wrT = np.ascontiguousarray((np.asarray(gcn_w_w)[:, :D] * inv).T, dtype=f32)
    weT = np.ascontiguousarray((np.asarray(gcn_w_w)[:, D:] * inv).T, dtype=f32)
    gcnb = np.broadcast_to(np.asarray(gcn_w_b) * (K / B), (P, D)).astype(f32)
    p1wT = np.ascontiguousarray(np.asarray(p1_w).T, dtype=f32)      # (D, 2D)
    p1b_col = np.ascontiguousarray(np.asarray(p1_b).reshape(2, P).T, dtype=f32)
    p2wT = np.ascontiguousarray(np.asarray(p2_w).T, dtype=f32)      # (2D, D)
    p2b_r = np.broadcast_to(np.asarray(p2_b), (P, D)).astype(f32)
    lna_r = np.broadcast_to(np.asarray(ln_a), (P, D)).astype(f32)
    lnb_r = np.broadcast_to(np.asarray(ln_b), (P, D)).astype(f32)
    wihT = np.ascontiguousarray(np.asarray(w_ih).T, dtype=f32)      # (D, 8D)
    whhT = np.ascontiguousarray(np.asarray(w_hh).T, dtype=f32)      # (2D, 8D)
    gbias = np.broadcast_to(np.asarray(b_ih) + np.asarray(b_hh),
                            (P, 8 * D)).astype(f32)
    common = {
        "table": table, "table16": table16, "wrT": wrT, "weT": weT,
        "gcnb": gcnb,
        "p1wT": p1wT, "p1b": p1b_col,
        "p2wTa": np.ascontiguousarray(p2wT[:D]),
        "p2wTb": np.ascontiguousarray(p2wT[D:]),
        "p2b": p2b_r, "lna": lna_r, "lnb": lnb_r,
        "wihT": wihT,
        "whhTa": np.ascontiguousarray(whhT[:D]),
        "whhTb": np.ascontiguousarray(whhT[D:]),
        "gbias": gbias,
    }
    in_maps = []
    for c in range(NCORES):
        rows = slice(c * P, (c + 1) * P)
        m = dict(common)
        m["idx"] = np.ascontiguousarray(
            np.concatenate([rel[rows], ent[rows]], axis=1))
        m["qidx"] = np.ascontiguousarray(qry[rows])
        in_maps.append(m)
    return in_maps


def _make(inputs):
    if "nc" not in _CACHE:
        _CACHE["nc"] = _build()
    return _CACHE["nc"], _prep_inputs(**inputs)


def kernel(**inputs) -> np.ndarray:
    nc, in_maps = _make(inputs)
    res = run_bass_kernel_spmd(nc, in_maps, list(range(NCORES)), trace=False)
    return np.concatenate([res.results[c]["out"][:, 0] for c in range(NCORES)])


# revision 16
# speedup vs baseline: 1.0055x; 1.0055x over previous
"""Trainium2 Bass kernel for nn_EmbedMatcher (GNN message passing).

Strategy: data-parallel over B=1024 across 8 cores (128 rows each); the
200001x128 symbol table is replicated per core (plus a bf16 copy staged for
the gather path, halving queue bytes). The neighbor gather is 400 [P,1]
indirect DMAs per core in bypass mode -- on this HW the Q7 SWDGE engine
generates one descriptor per gathered row at ~1.1us/128 rows, which is the
measured floor across every gather primitive; bypass (vs compute_op=add)
halves descriptor work, and the K-sum rides the otherwise-idle Vector
engine. The GCN linear is algebraically reordered: sum_k (concat @ W^T) ==
[rel_sum|ent_sum] @ W^T, so the big (B*K,256)@(256,128) matmul collapses to
(B,256)@(256,128). The loop-invariant q@W_ih gates and qT transpose are
computed inside the gather window. support_g is AllGathered in both layouts
([sg | sgT], one local pre-transpose instead of 8 post-collective ones) so
each core runs the LSTM attention (which couples the whole batch) on its
own 128 query rows; softmax skips max-subtraction (scores are O(+-30),
safe for f32 exp).

Alternatives measured this session and rejected:
- dma_gather (Q7 extended inst, int16 windowed indices over a
  pre-transformed table): ~10.4ns/row single-queue vs indirect's 8.8,
  plus ~60% zero-row padding from the 32768-row windows -> 1058us.
- dma_gather across 4 SWDGE queues (each queue = own Q7 cpu pair):
  2.2x gen throughput in isolation, but concurrent queue pairs corrupt
  gathered data in the full kernel (shared ucode read-port state), and
  per-engine descriptor rings cap one gather at 896 rows.
- multi-queue indirect DMA via qPoolDynamic{1..3} renaming: no speedup
  (mainline dynamic-DMA decode does not parallelize across queues).
"""
import numpy as np

from concourse import bass, bacc, mybir
import concourse.tile as tile
from concourse.bass_utils import run_bass_kernel_spmd

P = 128            # batch rows per core
D = 128            # embed dim
K = 200            # neighbors
NCORES = 8
NROWS = 200001     # symbol table rows (incl. padding row)
NCHAIN = 8         # parallel accumulate chains per table
STEPS = 4
F32 = mybir.dt.float32
I32 = mybir.dt.int32

_CACHE = {}


def _build():
    nc = bacc.Bacc("TRN2", target_bir_lowering=False, debug=False,
                   enable_asserts=True, num_devices=NCORES)
    ap = {}
    def inp(name, shape, dtype=F32):
        ap[name] = nc.dram_tensor(name, shape, dtype, kind="ExternalInput").ap()
    inp("table", [NROWS, D])
    inp("table16", [NROWS, D], mybir.dt.bfloat16)
    inp("idx", [P, 2 * K], I32)
    inp("qidx", [P, 1], I32)
    inp("wrT", [D, D]); inp("weT", [D, D]); inp("gcnb", [P, D])
    inp("p1wT", [D, 2 * D]); inp("p1b", [P, 2])
    inp("p2wTa", [D, D]); inp("p2wTb", [D, D]); inp("p2b", [P, D])
    inp("lna", [P, D]); inp("lnb", [P, D])
    inp("wihT", [D, 8 * D]); inp("whhTa", [D, 8 * D]); inp("whhTb", [D, 8 * D])
    inp("gbias", [P, 8 * D])
    out_d = nc.dram_tensor("out", [P, 1], F32, kind="ExternalOutput").ap()

    from concourse.masks import make_identity
    AX = mybir.AxisListType.X
    OP = mybir.AluOpType
    ACT = mybir.ActivationFunctionType

    with tile.TileContext(nc, num_cores=NCORES) as tc:
        with tc.tile_pool(name="sb", bufs=1) as sb, \
             tc.tile_pool(name="ps", bufs=2, space="PSUM") as ps, \
             tc.tile_pool(name="pst", bufs=2, space="PSUM") as pst, \
             tc.tile_pool(name="dram", bufs=1, space="DRAM") as dram:

            ident = sb.tile([P, P], F32)
            make_identity(nc, ident[:])

            # ---- load inputs to SBUF
            idx_sb = sb.tile([P, 2 * K], I32)
            nc.sync.dma_start(out=idx_sb[:], in_=ap["idx"][:])
            qidx_sb = sb.tile([P, 1], I32)
            nc.sync.dma_start(out=qidx_sb[:], in_=ap["qidx"][:])
            w = {}
            for name, shape in [("wrT", [D, D]), ("weT", [D, D]), ("gcnb", [P, D]),
                                ("p1wT", [D, 2 * D]), ("p1b", [P, 2]),
                                ("p2wTa", [D, D]), ("p2wTb", [D, D]), ("p2b", [P, D]),
                                ("lna", [P, D]), ("lnb", [P, D]),
                                ("wihT", [D, 8 * D]), ("whhTa", [D, 8 * D]),
                                ("whhTb", [D, 8 * D]), ("gbias", [P, 8 * D])]:
                w[name] = sb.tile(shape, F32, name=f"w_{name}")
                nc.sync.dma_start(out=w[name][:], in_=ap[name][:])

            def transpose_to(dst_sb, src_ap, nm):
                tp = pst.tile([P, P], F32, name=f"tp_{nm}", tag="tp")
                nc.tensor.transpose(out=tp[:], in_=src_ap, identity=ident[:])
                nc.vector.tensor_copy(out=dst_sb, in_=tp[:])

            # ---- query gather (B,) -> (P, D)
            q_sb = sb.tile([P, D], F32)
            nc.gpsimd.indirect_dma_start(
                out=q_sb[:], out_offset=None, in_=ap["table"][:],
                in_offset=bass.IndirectOffsetOnAxis(ap=qidx_sb[:, 0:1], axis=0))

            # ---- qT and q-gates: loop-invariant, computed during the
            # gather phase while TensorE is otherwise idle
            qT = sb.tile([P, P], F32)
            transpose_to(qT[:], q_sb[:], "q")
            gq = sb.tile([P, 8 * D], F32, name="gq")
            for j in range(2):
                gp = ps.tile([P, 512], F32, name=f"gq{j}", tag="gates")
                sl = slice(512 * j, 512 * (j + 1))
                nc.tensor.matmul(out=gp[:], lhsT=qT[:], rhs=w["wihT"][:, sl],
                                 start=True, stop=True)
                nc.vector.tensor_add(out=gq[:, sl], in0=gp[:],
                                     in1=w["gbias"][:, sl])

            # ---- neighbor gathers: [P,1] indirect DMAs (bypass) + VectorE
            # accumulate. On real HW an indirect DMA reads only ONE index per
            # partition (multi-column offset APs silently stream consecutive
            # rows), so the shape is forced to [P,1]. bypass halves the Q7
            # descriptor-gen work vs compute_op=add chains (RMW emits read
            # descriptors too); the K-sum rides the idle Vector engine.
            sums = []
            BF16 = mybir.dt.bfloat16
            bufs = [sb.tile([P, D], BF16, name=f"gbuf{i}")
                    for i in range(NCHAIN)]
            for half in range(2):          # 0 = relations, 1 = entities
                s = sb.tile([P, D], F32, name=f"sum{half}")
                for t in range(K):
                    col = half * K + t
                    b = bufs[t % NCHAIN]
                    nc.gpsimd.indirect_dma_start(
                        out=b[:], out_offset=None, in_=ap["table16"][:],
                        in_offset=bass.IndirectOffsetOnAxis(
                            ap=idx_sb[:, col:col + 1], axis=0))
                    if t == 0:
                        nc.vector.tensor_copy(out=s[:], in_=b[:])
                    else:
                        nc.vector.tensor_add(out=s[:], in0=s[:], in1=b[:])
                sums.append(s)
            rel_sum, ent_sum = sums

            # ---- GCN: support = tanh((rel_sum@Wr' + ent_sum@We') + gcnb)
            relT = sb.tile([P, P], F32); transpose_to(relT[:], rel_sum[:], "rel")
            entT = sb.tile([P, P], F32); transpose_to(entT[:], ent_sum[:], "ent")
            sup_ps = ps.tile([P, D], F32, name="sup_ps", tag="mm")
            nc.tensor.matmul(out=sup_ps[:], lhsT=relT[:], rhs=w["wrT"][:],
                             start=True, stop=False)
            nc.tensor.matmul(out=sup_ps[:], lhsT=entT[:], rhs=w["weT"][:],
                             start=False, stop=True)
            support = sb.tile([P, D], F32)
            nc.vector.tensor_add(out=support[:], in0=sup_ps[:], in1=w["gcnb"][:])
            nc.scalar.activation(out=support[:], in_=support[:], func=ACT.Tanh)

            # ---- FFN + residual + layernorm -> support_g
            supT = sb.tile([P, P], F32); transpose_to(supT[:], support[:], "sup")
            hidT = []
            for j in range(2):
                hp = ps.tile([P, D], F32, name=f"hid_ps{j}", tag="mm")
                nc.tensor.matmul(out=hp[:], lhsT=w["p1wT"][:, j * D:(j + 1) * D],
                                 rhs=supT[:], start=True, stop=True)
                ht = sb.tile([P, P], F32, name=f"hidT{j}")
                nc.scalar.activation(out=ht[:], in_=hp[:], func=ACT.Relu,
                                     bias=w["p1b"][:, j:j + 1])
                hidT.append(ht)
            o2 = ps.tile([P, D], F32, name="o2", tag="mm")
            nc.tensor.matmul(out=o2[:], lhsT=hidT[0][:], rhs=w["p2wTa"][:],
                             start=True, stop=False)
            nc.tensor.matmul(out=o2[:], lhsT=hidT[1][:], rhs=w["p2wTb"][:],
                             start=False, stop=True)
            z = sb.tile([P, D], F32)
            nc.vector.tensor_add(out=z[:], in0=o2[:], in1=support[:])
            nc.vector.tensor_add(out=z[:], in0=z[:], in1=w["p2b"][:])
            # layernorm (unbiased std, eps added to std)
            zsum = sb.tile([P, 1], F32)
            nc.vector.tensor_reduce(out=zsum[:], in_=z[:], axis=AX, op=OP.add)
            zmean = sb.tile([P, 1], F32)
            nc.scalar.mul(out=zmean[:], in_=zsum[:], mul=1.0 / D)
            xc = sb.tile([P, D], F32)
            nc.vector.tensor_scalar(out=xc[:], in0=z[:], scalar1=zmean[:, 0:1],
                                    scalar2=None, op0=OP.subtract)
            sqt = sb.tile([P, D], F32)
            varsum = sb.tile([P, 1], F32)
            nc.scalar.activation(out=sqt[:], in_=xc[:], func=ACT.Square,
                                 accum_out=varsum[:])
            sigma = sb.tile([P, 1], F32)
            nc.scalar.activation(out=sigma[:], in_=varsum[:], func=ACT.Sqrt,
                                 scale=1.0 / (D - 1))
            nc.vector.tensor_scalar(out=sigma[:], in0=sigma[:], scalar1=1e-3,
                                    scalar2=None, op0=OP.add)
            rec = sb.tile([P, 1], F32)
            nc.vector.reciprocal(out=rec[:], in_=sigma[:])
            sg = sb.tile([P, D], F32)
            nc.vector.tensor_scalar(out=sg[:], in0=xc[:], scalar1=rec[:, 0:1],
                                    scalar2=None, op0=OP.mult)
            nc.vector.tensor_tensor(out=sg[:], in0=sg[:], in1=w["lna"][:],
                                    op=OP.mult)
            nc.vector.tensor_tensor(out=sg[:], in0=sg[:], in1=w["lnb"][:],
                                    op=OP.add)

            # ---- AllGather [sg | sgT] -> both layouts on every core
            # (one local transpose pre-AG replaces 8 post-collective ones)
            sgT_own = sb.tile([P, P], F32)
            transpose_to(sgT_own[:], sg[:], "sgown")
            ag_in = dram.tile([P, 2 * D], F32)
            ag_out = dram.tile([NCORES * P, 2 * D], F32)
            nc.gpsimd.dma_start(out=ag_in[:, 0:D], in_=sg[:])
            nc.gpsimd.dma_start(out=ag_in[:, D:2 * D], in_=sgT_own[:])
            nc.gpsimd.collective_compute(
                "AllGather", OP.bypass,
                replica_groups=[list(range(NCORES))],
                ins=[ag_in.opt()], outs=[ag_out.opt()])
            sg_all = sb.tile([P, NCORES, D], F32)
            nc.sync.dma_start(
                out=sg_all[:],
                in_=ag_out[:, 0:D].rearrange("(c p) d -> p c d", c=NCORES))
            sgT3 = sb.tile([P, NCORES, P], F32)
            nc.sync.dma_start(
                out=sgT3[:],
                in_=ag_out[:, D:2 * D].rearrange("(c d) b -> d c b",
                                                 c=NCORES))
            sgT = sgT3[:].rearrange("d c b -> d (c b)")

            # ---- LSTM + attention
            c_sb = sb.tile([P, 2 * D], F32)
            gts = sb.tile([P, 8 * D], F32)
            si = sb.tile([P, 2 * D], F32)
            sf = sb.tile([P, 2 * D], F32)
            tg = sb.tile([P, 2 * D], F32)
            so = sb.tile([P, D], F32)
            tch = sb.tile([P, D], F32)
            ho = sb.tile([P, D], F32)
            hoT = sb.tile([P, P], F32)
            rT_sb = sb.tile([P, P], F32)
            attn = sb.tile([P, NCORES * P], F32)
            rmax = sb.tile([P, 1], F32)
            negmax = sb.tile([P, 1], F32)
            rowsum = sb.tile([P, 1], F32)
            rsrec = sb.tile([P, 1], F32)

            for s in range(STEPS):
                if s == 0:
                    gsrc = gq
                else:
                    gsrc = gts
                    for j in range(2):  # gate halves: g-slices [512j, 512j+512)
                        gp = ps.tile([P, 512], F32, name=f"g{s}{j}", tag="gates")
                        sl = slice(512 * j, 512 * (j + 1))
                        nc.tensor.matmul(out=gp[:], lhsT=hoT[:],
                                         rhs=w["whhTa"][:, sl],
                                         start=True, stop=False)
                        nc.tensor.matmul(out=gp[:], lhsT=rT_sb[:],
                                         rhs=w["whhTb"][:, sl],
                                         start=False, stop=True)
                        nc.vector.tensor_add(out=gts[:, sl], in0=gp[:],
                                             in1=gq[:, sl])
                # i,f,g,o = gsrc[0:256],[256:512],[512:768],[768:1024]
                nc.scalar.activation(out=si[:], in_=gsrc[:, 0:256], func=ACT.Sigmoid)
                nc.scalar.activation(out=tg[:], in_=gsrc[:, 512:768], func=ACT.Tanh)
                nc.scalar.activation(out=so[:], in_=gsrc[:, 768:896], func=ACT.Sigmoid)
                if s == 0:
                    nc.vector.tensor_tensor(out=c_sb[:], in0=si[:], in1=tg[:],
                                            op=OP.mult)
                else:
                    nc.scalar.activation(out=sf[:], in_=gsrc[:, 256:512],
                                         func=ACT.Sigmoid)
                    nc.vector.tensor_tensor(out=sf[:], in0=sf[:], in1=c_sb[:],
                                            op=OP.mult)
                    nc.vector.tensor_tensor(out=si[:], in0=si[:], in1=tg[:],
                                            op=OP.mult)
                    nc.vector.tensor_add(out=c_sb[:], in0=sf[:], in1=si[:])
                nc.scalar.activation(out=tch[:], in_=c_sb[:, 0:D], func=ACT.Tanh)
                nc.vector.tensor_tensor(out=tch[:], in0=so[:], in1=tch[:],
                                        op=OP.mult)
                nc.vector.tensor_add(out=ho[:], in0=q_sb[:], in1=tch[:])
                if s == STEPS - 1:
                    break
                transpose_to(hoT[:], ho[:], f"ho{s}")
                sc = ps.tile([P, NCORES * P], F32, name=f"sc{s}", tag="scores",
                             bufs=1)
                for j in range(2):
                    nc.tensor.matmul(out=sc[:, 512 * j:512 * (j + 1)],
                                     lhsT=hoT[:],
                                     rhs=sgT[:, 512 * j:512 * (j + 1)],
                                     start=True, stop=True)
                # scores are O(+-30): exp is safe in f32 without the
                # max-subtraction, saving a 1024-wide reduce per step
                nc.scalar.activation(out=attn[:], in_=sc[:], func=ACT.Exp,
                                     accum_out=rowsum[:])
                nc.vector.reciprocal(out=rsrec[:], in_=rowsum[:])
                nc.vector.tensor_scalar(out=attn[:], in0=attn[:],
                                        scalar1=rsrec[:, 0:1], scalar2=None,
                                        op0=OP.mult)
                rp = ps.tile([P, D], F32, name=f"rp{s}", tag="mm")
                for c in range(NCORES):
                    at = sb.tile([P, P], F32, name=f"at{s}{c}", tag="atT",
                                 bufs=2)
                    transpose_to(at[:], attn[:, c * P:(c + 1) * P], f"at{s}{c}")
                    nc.tensor.matmul(out=rp[:], lhsT=sg_all[:, c, :], rhs=at[:],
                                     start=(c == 0), stop=(c == NCORES - 1))
                nc.vector.tensor_copy(out=rT_sb[:], in_=rp[:])

            # ---- cosine similarity against own support_g shard
            m1 = sb.tile([P, D], F32)
            nc.vector.tensor_tensor(out=m1[:], in0=ho[:], in1=sg[:], op=OP.mult)
            cross = sb.tile([P, 1], F32)
            nc.vector.tensor_reduce(out=cross[:], in_=m1[:], axis=AX, op=OP.add)
            n1 = sb.tile([P, 1], F32)
            n2 = sb.tile([P, 1], F32)
            nc.scalar.activation(out=m1[:], in_=ho[:], func=ACT.Square,
                                 accum_out=n1[:])
            nc.scalar.activation(out=m1[:], in_=sg[:], func=ACT.Square,
                                 accum_out=n2[:])
            nc.vector.tensor_tensor(out=n1[:], in0=n1[:], in1=n2[:], op=OP.mult)
            nc.scalar.activation(out=n1[:], in_=n1[:], func=ACT.Sqrt)
            nc.vector.reciprocal(out=n1[:], in_=n1[:])
            res = sb.tile([P, 1], F32)
            nc.vector.tensor_tensor(out=res[:], in0=cross[:], in1=n1[:],
                                    op=OP.mult)
            nc.sync.dma_start(out=out_d[:], in_=res[:])
    nc.compile()
    return nc


def _prep_inputs(relations, entities, query, symbol_emb, gcn_w_w, gcn_w_b,
                 p1_w, p1_b, p2_w, p2_b, ln_a, ln_b, w_ih, w_hh, b_ih, b_hh):
    import ml_dtypes
    f32 = np.float32
    table = np.ascontiguousarray(symbol_emb, dtype=f32)
    table16 = table.astype(ml_dtypes.bfloat16)
    B = relations.shape[0]
    rel = np.asarray(relations).astype(np.int32)
    ent = np.asarray(entities).astype(np.int32)
    qry = np.asarray(query).astype(np.int32).reshape(B, 1)
    inv = f32(1.0 / B)                     # reference divides by B (quirk)
    wrT = np.ascontiguousarray((np.asarray(gcn_w_w)[:, :D] * inv).T, dtype=f32)
    weT = np.ascontiguousarray((np.asarray(gcn_w_w)[:, D:] * inv).T, dtype=f32)
    gcnb = np.broadcast_to(np.asarray(gcn_w_b) * (K / B), (P, D)).astype(f32)
    p1wT = np.ascontiguousarray(np.asarray(p1_w).T, dtype=f32)      # (D, 2D)
    p1b_col = np.ascontiguousarray(np.asarray(p1_b).reshape(2, P).T, dtype=f32)
    p2wT = np.ascontiguousarray(np.asarray(p2_w).T, dtype=f32)      # (2D, D)
    p2b_r = np.broadcast_to(np.asarray(p2_b), (P, D)).astype(f32)
    lna_r = np.broadcast_to(np.asarray(ln_a), (P, D)).astype(f32)
    lnb_r = np.broadcast_to(np.asarray(ln_b), (P, D)).astype(f32)
    wihT = np.ascontiguousarray(np.asarray(w_ih).T, dtype=f32)      # (D, 8D)
    whhT = np.ascontiguousarray(np.asarray(w_hh).T, dtype=f32)      # (2D, 8D)
    gbias = np.broadcast_to(np.asarray(b_ih) + np.asarray(b_hh),
                            (P, 8 * D)).astype(f32)
    common = {
        "table": table, "table16": table16, "wrT": wrT, "weT": weT,
        "gcnb": gcnb,
        "p1wT": p1wT, "p1b": p1b_col,
        "p2wTa": np.ascontiguousarray(p2wT[:D]),
        "p2wTb": np.ascontiguousarray(p2wT[D:]),
        "p2b": p2b_r, "lna": lna_r, "lnb": lnb_r,
        "wihT": wihT,
        "whhTa": np.ascontiguousarray(whhT[:D]),
        "whhTb": np.ascontiguousarray(whhT[D:]),
        "gbias": gbias,
    }
    in_maps = []
    for c in range(NCORES):
        rows = slice(c * P, (c + 1) * P)
        m = dict(common)
        m["idx"] = np.ascontiguousarray(
            np.concatenate([rel[rows], ent[rows]], axis=1))
        m["qidx"] = np.ascontiguousarray(qry[rows])
        in_maps.append(m)
    return in_maps


def _make(inputs):
    if "nc" not in _CACHE:
        _CACHE["nc"] = _build()
    return _CACHE["nc"], _prep_inputs(**inputs)


def kernel(**inputs) -> np.ndarray:
    nc, in_maps = _make(inputs)
    res = run_bass_kernel_spmd(nc, in_maps, list(range(NCORES)), trace=False)
    return np.concatenate([res.results[c]["out"][:, 0] for c in range(NCORES)])
